# revision 32
# baseline (speedup 1.0000x reference)
"""Trainium2 Bass kernel for nn_DecodePredictions (YOLO-style decode, B=16).

Pure data-parallel over batch (2 images per core x 8 cores).

The [B, N*C, 6] output is hugely redundant on the device side: per anchor,
the 4 box coords repeat across all 80 classes and lane 4 is the constant
class id. The device therefore emits only the per-anchor uniques --
4 box coords (bf16) and 80 class scores (bf16) -- ~1.5 MB/core instead of
the 16 MB/core the full layout costs; the host broadcasts them into the
full [B, N*C, 6] fp32 array while unsharding.

Scores: sigma on ACT (the only engine with activation LUTs), then one
tensor_tensor multiply per class chunk against broadcast sigma(obj) --
all-bf16 step-1 APs keep the DVE in its 2x packed mode (the fused
scalar_tensor_tensor alternative only has a 1x uop and measures 2x
slower). Box wh avoids the Exp table entirely -- exp(w) =
sigma(w)/(1-sigma(w)) via the DVE's hardware-divide reciprocal on the
tiny [P, 264] plane -- so both ACT_TABLE_LOADs finish during the DMA
latency of the first preds chunk and never gate the sigmoid ladder.

Layouts are class-major [P, 81, KPP] (anchor innermost) so every ACT/DVE
op is step-1 contiguous and every DMA moves 128 contiguous per-partition
segments of 2-5 KB. All DMAs ride the sync HWDGE ring, keeping the
scalar sequencer a pure ACT stream (descriptor gen is ~600ns a piece
and would delay activation dispatch); input chunks are sized so each
sigma's data lands just before the ladder reaches it, score chunks
stream out as their multiplies finish, boxes slot into the out ring
mid-stream, and the trailing class chunks shrink (12/8/5 rows) so the
final sigma->multiply->DMA->receipt chain is short. The only foreign
work on GpSimd is the pure-f32 box xy pair (its mixed-dtype ops run
2.6x slower and contend with DVE SBUF ports).

The kernel is raw bacc with hand-placed semaphores -- no TileContext.
Tile's entry/exit barrier ceremony and scheduler slack cost a measured
~2.5-4us on a kernel this small. Each DMA then_inc's its OWN semaphore
by 16 (one inc per SDMA engine): a shared counter with cumulative
thresholds races, because the 16 engines' increments skew across DMAs,
so sem >= 16k does NOT imply the first k DMAs finished. The one shared
output counter is only waited at its maximum (128 = all 8 DMAs), which
is skew-safe. Compute producers then_inc(+1) per-engine counters that
single-step in program order.
"""

import ml_dtypes
import numpy as np

N_CORES = 8
B = 16
B_PER_CORE = B // N_CORES  # 2
C = 80
F = 85
N_REAL = 8400              # 80*80 + 40*40 + 20*20
N_PAD = 8448               # = 66 * 128
P = 128
KPP = B_PER_CORE * N_PAD // P  # 132 anchors per partition
R = C + 1                  # obj row + 80 class rows
# sigma-row chunks [r0, r1); chunk 0 carries the obj row. Score rows are
# the same ranges shifted down by one, so multiply chunk c depends only on
# sigma chunk c (plus sigma(obj) from chunk 0). Leading chunks are sized
# so each input DMA's completion receipt (~2us after its transfer) lands
# just before the sigma ladder reaches it -- no receipt gaps; trailing
# chunks shrink so the final sigma->mul->DMA->receipt drain is short.
SCH = [(0, 12), (12, 24), (24, 38), (38, 56), (56, 68), (68, 76), (76, 81)]
# input DMA row chunks; sigma chunk c waits on input sem c (earlier sems
# are already waited cumulatively on the ACT stream, so a sigma chunk
# spanning a DMA boundary is still safe)
DCH = [(0, 12), (12, 24), (24, 42), (42, 60), (60, 81)]

_CACHE: dict = {}


def _build_nc():
    import concourse.bacc as bacc
    from concourse import mybir
    from contextlib import ExitStack

    nc = bacc.Bacc("TRN2", target_bir_lowering=False, debug=False)
    predsT = nc.dram_tensor("predsT", [P, R, KPP], mybir.dt.float8e4, kind="ExternalInput")
    pa = nc.dram_tensor("pa", [P, 4, KPP], mybir.dt.float32, kind="ExternalInput")
    aux = nc.dram_tensor("aux", [P, 4, KPP], mybir.dt.bfloat16, kind="ExternalInput")
    scores = nc.dram_tensor("scores", [P, C, KPP], mybir.dt.bfloat16, kind="ExternalOutput")
    boxes = nc.dram_tensor("boxes", [P, 4, KPP], mybir.dt.bfloat16, kind="ExternalOutput")

    fp32 = mybir.dt.float32
    bf16 = mybir.dt.bfloat16
    AF = mybir.ActivationFunctionType
    OP = mybir.AluOpType

    with ExitStack() as ctx:
        sem = lambda n: ctx.enter_context(nc.semaphore(n))
        sb = lambda n, s, d: ctx.enter_context(nc.sbuf_tensor(n, s, d))

        (s_c0, s_pa, s_c1, s_c2, s_aux, s_c3, s_c4) = [
            sem(f"in{i}") for i in range(7)
        ]
        act_sem = sem("act_sem")
        mul_sem = sem("mul_sem")
        box_sem = sem("box_sem")
        gp_sem = sem("gp_sem")
        out_sem = sem("out_sem")

        pt = sb("pt", [P, R, KPP], mybir.dt.float8e4)
        pa_t = sb("pa_t", [P, 4, KPP], fp32)
        aux_t = sb("aux_t", [P, 4, KPP], bf16)
        sg = sb("sg", [P, R, KPP], bf16)
        sp = sb("sp", [P, 2, KPP], fp32)
        sc_t = sb("sc_t", [P, C, KPP], bf16)
        om_t = sb("om_t", [P, 2, KPP], fp32)
        rc_t = sb("rc_t", [P, 2, KPP], fp32)
        wh_t = sb("wh_t", [P, 2, KPP], fp32)
        xy_s = sb("xy_s", [P, 2, KPP], fp32)
        xy1 = sb("xy1", [P, 2, KPP], fp32)
        wh_s = sb("wh_s", [P, 2, KPP], fp32)
        box_t = sb("box_t", [P, 4, KPP], bf16)

        # ---- SP stream: inputs (order c0, pa, c1, c2, aux, c3, c4) ----
        def din(s, dst, src):
            nc.sync.dma_start(dst, src).then_inc(s, 16)

        din(s_c0, pt[:, DCH[0][0] : DCH[0][1], :], predsT[:, DCH[0][0] : DCH[0][1], :])
        din(s_pa, pa_t[:], pa[:])
        din(s_c1, pt[:, DCH[1][0] : DCH[1][1], :], predsT[:, DCH[1][0] : DCH[1][1], :])
        din(s_c2, pt[:, DCH[2][0] : DCH[2][1], :], predsT[:, DCH[2][0] : DCH[2][1], :])
        din(s_aux, aux_t[:], aux[:])
        din(s_c3, pt[:, DCH[3][0] : DCH[3][1], :], predsT[:, DCH[3][0] : DCH[3][1], :])
        din(s_c4, pt[:, DCH[4][0] : DCH[4][1], :], predsT[:, DCH[4][0] : DCH[4][1], :])

        # ---- ACT stream: sigma ladder ----
        def sig(c):
            r0, r1 = SCH[c]
            return nc.scalar.activation(sg[:, r0:r1, :], pt[:, r0:r1, :], AF.Sigmoid)

        nc.scalar.wait_ge(s_c0, 16)
        sig(0).then_inc(act_sem, 1)                      # act=1
        nc.scalar.wait_ge(s_pa, 16)
        nc.scalar.activation(sp[:], pa_t[:, 2:4, :], AF.Sigmoid).then_inc(act_sem, 1)  # act=2
        nc.scalar.wait_ge(s_c1, 16)
        sig(1).then_inc(act_sem, 1)                      # act=3
        nc.scalar.wait_ge(s_c2, 16)
        sig(2).then_inc(act_sem, 1)                      # act=4
        nc.scalar.wait_ge(s_c3, 16)
        sig(3).then_inc(act_sem, 1)                      # act=5
        nc.scalar.wait_ge(s_c4, 16)
        sig(4).then_inc(act_sem, 1)                      # act=6
        sig(5).then_inc(act_sem, 1)                      # act=7
        sig(6).then_inc(act_sem, 1)                      # act=8

        # ---- GP stream: xy path ----
        nc.gpsimd.wait_ge(s_pa, 16)
        nc.gpsimd.wait_ge(s_aux, 16)
        nc.gpsimd.tensor_mul(xy_s[:], pa_t[:, 0:2, :], aux_t[:, 0:2, :])
        nc.gpsimd.tensor_add(xy1[:], xy_s[:], aux_t[:, 2:4, :]).then_inc(gp_sem, 1)

        # ---- DVE stream ----
        # act_sem order: sig0=1, sigbox=2, sig1=3, sig2=4, ..., sig6=8
        def mul(c):
            r0, r1 = SCH[c]
            s0, t0 = (0, 1) if c == 0 else (r0 - 1, r0)
            s1 = r1 - 1
            nc.vector.wait_ge(act_sem, c + 2 if c else 1)
            nc.vector.tensor_mul(
                sc_t[:, s0:s1, :],
                sg[:, t0:r1, :],
                sg[:, 0, :].unsqueeze(1).broadcast_to([P, s1 - s0, KPP]),
            ).then_inc(mul_sem, 1)

        mul(0)
        nc.vector.wait_ge(act_sem, 2)
        nc.vector.tensor_scalar(om_t[:], sp[:], 1.0, -1.0, OP.subtract, OP.mult)
        nc.vector.reciprocal(rc_t[:], om_t[:])
        mul(1)
        nc.vector.tensor_mul(wh_t[:], sp[:], rc_t[:])
        nc.vector.wait_ge(s_aux, 16)
        nc.vector.tensor_mul(wh_s[:], wh_t[:], aux_t[:, 0:2, :])
        mul(2)
        nc.vector.wait_ge(gp_sem, 1)
        nc.vector.tensor_copy(box_t[:, 0:2, :], xy1[:])
        nc.vector.tensor_add(box_t[:, 2:4, :], xy1[:], wh_s[:]).then_inc(box_sem, 1)
        for c in range(3, len(SCH)):
            mul(c)

        # ---- SP stream: outputs ----
        def dout(c):
            r0, r1 = SCH[c]
            s0 = 0 if c == 0 else r0 - 1
            s1 = r1 - 1
            nc.sync.wait_ge(mul_sem, c + 1)
            nc.sync.dma_start(scores[:, s0:s1, :], sc_t[:, s0:s1, :]).then_inc(out_sem, 16)

        dout(0)
        dout(1)
        dout(2)
        nc.sync.wait_ge(box_sem, 1)
        nc.sync.dma_start(boxes[:], box_t[:]).then_inc(out_sem, 16)
        for c in range(3, len(SCH)):
            dout(c)
        # final visibility: all 8 output DMAs confirmed in HBM
        nc.sync.wait_ge(out_sem, 16 * 8)

    nc.compile()
    return nc


def _host_consts():
    # Per-anchor stride s and grid offsets bx = gx*s, by = gy*s, padded to
    # N_PAD, replicated for the 2 images per core, as [P, 4, KPP] planes
    # (s, s, bx, by). All values are exact in bf16.
    s = np.ones(N_PAD, np.float32)
    bx = np.zeros(N_PAD, np.float32)
    by = np.zeros(N_PAD, np.float32)
    off = 0
    for g, st in ((80, 8.0), (40, 16.0), (20, 32.0)):
        n = g * g
        i = np.arange(n)
        s[off : off + n] = st
        bx[off : off + n] = (i % g) * st
        by[off : off + n] = (i // g) * st
        off += n
    pl = np.stack([s, s, bx, by], 0)                     # [4, N_PAD]
    pl = np.concatenate([pl] * B_PER_CORE, 1)            # [4, 2*N_PAD]
    aux = pl.reshape(4, P, KPP).transpose(1, 0, 2)       # [P, 4, KPP]
    return np.ascontiguousarray(aux.astype(ml_dtypes.bfloat16))


def _host_in_maps(pred0, pred1, pred2):
    aux = _CACHE["consts"]
    pred0 = np.asarray(pred0, np.float32).reshape(B, -1, F)
    pred1 = np.asarray(pred1, np.float32).reshape(B, -1, F)
    pred2 = np.asarray(pred2, np.float32).reshape(B, -1, F)
    in_maps = []
    for core in range(N_CORES):
        flat = np.zeros((B_PER_CORE * N_PAD, F), np.float32)
        for j in range(B_PER_CORE):
            b = core * B_PER_CORE + j
            flat[j * N_PAD : j * N_PAD + N_REAL] = np.concatenate(
                [pred0[b], pred1[b], pred2[b]], axis=0
            )
        a = flat.reshape(P, KPP, F)                      # [p, k, field]
        predsT = np.empty((P, R, KPP), np.float32)
        predsT[:, 0, :] = a[:, :, 4]
        predsT[:, 1:, :] = a[:, :, 5:].transpose(0, 2, 1)
        in_maps.append(
            {
                "predsT": predsT.astype(ml_dtypes.float8_e4m3fn),
                "pa": np.ascontiguousarray(a[:, :, 0:4].transpose(0, 2, 1)),
                "aux": aux,
            }
        )
    return in_maps


def kernel(images, pred0, pred1, pred2):
    from concourse.bass_utils import run_bass_kernel_spmd

    if "nc" not in _CACHE:
        _CACHE["consts"] = _host_consts()
        _CACHE["nc"] = _build_nc()
    nc = _CACHE["nc"]

    in_maps = _host_in_maps(pred0, pred1, pred2)
    res = run_bass_kernel_spmd(nc, in_maps, list(range(N_CORES)))

    final = np.empty((B, N_REAL * C, 6), np.float32)
    v = final.reshape(B, N_REAL, C, 6)
    v[..., 4] = np.arange(C, dtype=np.float32)[None, None, :]
    for core, r in enumerate(res.results):
        # [P, C, KPP] -> per-image [N_REAL, C]; [P, 4, KPP] -> [N_REAL, 4]
        sc = (
            r["scores"].astype(np.float32)
            .reshape(B_PER_CORE, P // B_PER_CORE, C, KPP)
            .transpose(0, 1, 3, 2)
            .reshape(B_PER_CORE, N_PAD, C)
        )
        bx = (
            r["boxes"].astype(np.float32)
            .reshape(B_PER_CORE, P // B_PER_CORE, 4, KPP)
            .transpose(0, 1, 3, 2)
            .reshape(B_PER_CORE, N_PAD, 4)
        )
        for j in range(B_PER_CORE):
            b = core * B_PER_CORE + j
            v[b, :, :, 0:4] = bx[j, :N_REAL, None, :]
            v[b, :, :, 5] = sc[j, :N_REAL, :]
    return final


# revision 33
# speedup vs baseline: 1.0117x; 1.0117x over previous
"""Trainium2 Bass kernel for nn_DecodePredictions (YOLO-style decode, B=16).

Pure data-parallel over batch (2 images per core x 8 cores).

The [B, N*C, 6] output is hugely redundant on the device side: per anchor,
the 4 box coords repeat across all 80 classes and lane 4 is the constant
class id. The device therefore emits only the per-anchor uniques --
4 box coords (bf16) and 80 class scores (bf16) -- ~1.5 MB/core instead of
the 16 MB/core the full layout costs; the host broadcasts them into the
full [B, N*C, 6] fp32 array while unsharding.

Scores: sigma on ACT (the only engine with activation LUTs), then one
tensor_tensor multiply per class chunk against broadcast sigma(obj) --
all-bf16 step-1 APs keep the DVE in its 2x packed mode (the fused
scalar_tensor_tensor alternative only has a 1x uop and measures 2x
slower). Box wh avoids the Exp table entirely -- exp(w) =
sigma(w)/(1-sigma(w)) via the DVE's hardware-divide reciprocal on the
tiny [P, 264] plane -- so both ACT_TABLE_LOADs finish during the DMA
latency of the first preds chunk and never gate the sigmoid ladder.

Layouts are class-major [P, 81, KPP] (anchor innermost) so every ACT/DVE
op is step-1 contiguous and every DMA moves 128 contiguous per-partition
segments of 2-5 KB. All DMAs ride the sync HWDGE ring, keeping the
scalar sequencer a pure ACT stream (descriptor gen is ~600ns a piece
and would delay activation dispatch); input chunks are sized so each
sigma's data lands just before the ladder reaches it, score chunks
stream out as their multiplies finish, boxes slot into the out ring
mid-stream, and the trailing class chunks shrink (12/8/5 rows) so the
final sigma->multiply->DMA->receipt chain is short. The only foreign
work on GpSimd is the pure-f32 box xy pair (its mixed-dtype ops run
2.6x slower and contend with DVE SBUF ports).

The kernel is raw bacc with hand-placed semaphores -- no TileContext.
Tile's entry/exit barrier ceremony and scheduler slack cost a measured
~2.5-4us on a kernel this small. Each DMA then_inc's its OWN semaphore
by 16 (one inc per SDMA engine): a shared counter with cumulative
thresholds races, because the 16 engines' increments skew across DMAs,
so sem >= 16k does NOT imply the first k DMAs finished. The one shared
output counter is only waited at its maximum (128 = all 8 DMAs), which
is skew-safe. Compute producers then_inc(+1) per-engine counters that
single-step in program order.
"""

import ml_dtypes
import numpy as np

N_CORES = 8
B = 16
B_PER_CORE = B // N_CORES  # 2
C = 80
F = 85
N_REAL = 8400              # 80*80 + 40*40 + 20*20
N_PAD = 8448               # = 66 * 128
P = 128
KPP = B_PER_CORE * N_PAD // P  # 132 anchors per partition
R = C + 1                  # obj row + 80 class rows
# sigma-row chunks [r0, r1); chunk 0 carries the obj row. Score rows are
# the same ranges shifted down by one, so multiply chunk c depends only on
# sigma chunk c (plus sigma(obj) from chunk 0). Leading chunks are sized
# so each input DMA's completion receipt (~2us after its transfer) lands
# just before the sigma ladder reaches it -- no receipt gaps; trailing
# chunks shrink so the final sigma->mul->DMA->receipt drain is short.
SCH = [(0, 12), (12, 24), (24, 42), (42, 56), (56, 68), (68, 76), (76, 81)]
# input DMA row chunks; sigma chunk c waits on input sem c (earlier sems
# are already waited cumulatively on the ACT stream, so a sigma chunk
# spanning a DMA boundary is still safe)
DCH = [(0, 12), (12, 24), (24, 42), (42, 60), (60, 81)]

_CACHE: dict = {}


def _build_nc():
    import concourse.bacc as bacc
    from concourse import mybir
    from contextlib import ExitStack

    nc = bacc.Bacc("TRN2", target_bir_lowering=False, debug=False)
    predsT = nc.dram_tensor("predsT", [P, R, KPP], mybir.dt.float8e4, kind="ExternalInput")
    pa = nc.dram_tensor("pa", [P, 4, KPP], mybir.dt.float32, kind="ExternalInput")
    aux = nc.dram_tensor("aux", [P, 4, KPP], mybir.dt.bfloat16, kind="ExternalInput")
    scores = nc.dram_tensor("scores", [P, C, KPP], mybir.dt.bfloat16, kind="ExternalOutput")
    boxes = nc.dram_tensor("boxes", [P, 4, KPP], mybir.dt.bfloat16, kind="ExternalOutput")

    fp32 = mybir.dt.float32
    bf16 = mybir.dt.bfloat16
    AF = mybir.ActivationFunctionType
    OP = mybir.AluOpType

    with ExitStack() as ctx:
        sem = lambda n: ctx.enter_context(nc.semaphore(n))
        sb = lambda n, s, d: ctx.enter_context(nc.sbuf_tensor(n, s, d))

        (s_c0, s_pa, s_c1, s_c2, s_aux, s_c3, s_c4) = [
            sem(f"in{i}") for i in range(7)
        ]
        act_sem = sem("act_sem")
        mul_sem = sem("mul_sem")
        box_sem = sem("box_sem")
        gp_sem = sem("gp_sem")
        out_sem = sem("out_sem")

        pt = sb("pt", [P, R, KPP], mybir.dt.float8e4)
        pa_t = sb("pa_t", [P, 4, KPP], fp32)
        aux_t = sb("aux_t", [P, 4, KPP], bf16)
        sg = sb("sg", [P, R, KPP], bf16)
        sp = sb("sp", [P, 2, KPP], fp32)
        sc_t = sb("sc_t", [P, C, KPP], bf16)
        om_t = sb("om_t", [P, 2, KPP], fp32)
        rc_t = sb("rc_t", [P, 2, KPP], fp32)
        wh_t = sb("wh_t", [P, 2, KPP], fp32)
        xy_s = sb("xy_s", [P, 2, KPP], fp32)
        xy1 = sb("xy1", [P, 2, KPP], fp32)
        wh_s = sb("wh_s", [P, 2, KPP], fp32)
        box_t = sb("box_t", [P, 4, KPP], bf16)

        # ---- SP stream: inputs (order c0, pa, c1, c2, aux, c3, c4) ----
        def din(s, dst, src):
            nc.sync.dma_start(dst, src).then_inc(s, 16)

        din(s_c0, pt[:, DCH[0][0] : DCH[0][1], :], predsT[:, DCH[0][0] : DCH[0][1], :])
        din(s_pa, pa_t[:], pa[:])
        din(s_c1, pt[:, DCH[1][0] : DCH[1][1], :], predsT[:, DCH[1][0] : DCH[1][1], :])
        din(s_c2, pt[:, DCH[2][0] : DCH[2][1], :], predsT[:, DCH[2][0] : DCH[2][1], :])
        din(s_aux, aux_t[:], aux[:])
        din(s_c3, pt[:, DCH[3][0] : DCH[3][1], :], predsT[:, DCH[3][0] : DCH[3][1], :])
        din(s_c4, pt[:, DCH[4][0] : DCH[4][1], :], predsT[:, DCH[4][0] : DCH[4][1], :])

        # ---- ACT stream: sigma ladder ----
        def sig(c):
            r0, r1 = SCH[c]
            return nc.scalar.activation(sg[:, r0:r1, :], pt[:, r0:r1, :], AF.Sigmoid)

        nc.scalar.wait_ge(s_c0, 16)
        sig(0).then_inc(act_sem, 1)                      # act=1
        nc.scalar.wait_ge(s_pa, 16)
        nc.scalar.activation(sp[:], pa_t[:, 2:4, :], AF.Sigmoid).then_inc(act_sem, 1)  # act=2
        nc.scalar.wait_ge(s_c1, 16)
        sig(1).then_inc(act_sem, 1)                      # act=3
        nc.scalar.wait_ge(s_c2, 16)
        sig(2).then_inc(act_sem, 1)                      # act=4
        nc.scalar.wait_ge(s_c3, 16)
        sig(3).then_inc(act_sem, 1)                      # act=5
        nc.scalar.wait_ge(s_c4, 16)
        sig(4).then_inc(act_sem, 1)                      # act=6
        sig(5).then_inc(act_sem, 1)                      # act=7
        sig(6).then_inc(act_sem, 1)                      # act=8

        # ---- GP stream: xy path ----
        nc.gpsimd.wait_ge(s_pa, 16)
        nc.gpsimd.wait_ge(s_aux, 16)
        nc.gpsimd.tensor_mul(xy_s[:], pa_t[:, 0:2, :], aux_t[:, 0:2, :])
        nc.gpsimd.tensor_add(xy1[:], xy_s[:], aux_t[:, 2:4, :]).then_inc(gp_sem, 1)

        # ---- DVE stream ----
        # act_sem order: sig0=1, sigbox=2, sig1=3, sig2=4, ..., sig6=8
        def mul(c):
            r0, r1 = SCH[c]
            s0, t0 = (0, 1) if c == 0 else (r0 - 1, r0)
            s1 = r1 - 1
            nc.vector.wait_ge(act_sem, c + 2 if c else 1)
            nc.vector.tensor_mul(
                sc_t[:, s0:s1, :],
                sg[:, t0:r1, :],
                sg[:, 0, :].unsqueeze(1).broadcast_to([P, s1 - s0, KPP]),
            ).then_inc(mul_sem, 1)

        mul(0)
        nc.vector.wait_ge(act_sem, 2)
        nc.vector.tensor_scalar(om_t[:], sp[:], 1.0, -1.0, OP.subtract, OP.mult)
        nc.vector.reciprocal(rc_t[:], om_t[:])
        mul(1)
        nc.vector.tensor_mul(wh_t[:], sp[:], rc_t[:])
        nc.vector.wait_ge(s_aux, 16)
        nc.vector.tensor_mul(wh_s[:], wh_t[:], aux_t[:, 0:2, :])
        mul(2)
        nc.vector.wait_ge(gp_sem, 1)
        nc.vector.tensor_copy(box_t[:, 0:2, :], xy1[:])
        nc.vector.tensor_add(box_t[:, 2:4, :], xy1[:], wh_s[:]).then_inc(box_sem, 1)
        for c in range(3, len(SCH)):
            mul(c)

        # ---- SP stream: outputs ----
        def dout(c):
            r0, r1 = SCH[c]
            s0 = 0 if c == 0 else r0 - 1
            s1 = r1 - 1
            nc.sync.wait_ge(mul_sem, c + 1)
            nc.sync.dma_start(scores[:, s0:s1, :], sc_t[:, s0:s1, :]).then_inc(out_sem, 16)

        dout(0)
        dout(1)
        dout(2)
        nc.sync.wait_ge(box_sem, 1)
        nc.sync.dma_start(boxes[:], box_t[:]).then_inc(out_sem, 16)
        for c in range(3, len(SCH)):
            dout(c)
        # final visibility: all 8 output DMAs confirmed in HBM
        nc.sync.wait_ge(out_sem, 16 * 8)

    nc.compile()
    return nc


def _host_consts():
    # Per-anchor stride s and grid offsets bx = gx*s, by = gy*s, padded to
    # N_PAD, replicated for the 2 images per core, as [P, 4, KPP] planes
    # (s, s, bx, by). All values are exact in bf16.
    s = np.ones(N_PAD, np.float32)
    bx = np.zeros(N_PAD, np.float32)
    by = np.zeros(N_PAD, np.float32)
    off = 0
    for g, st in ((80, 8.0), (40, 16.0), (20, 32.0)):
        n = g * g
        i = np.arange(n)
        s[off : off + n] = st
        bx[off : off + n] = (i % g) * st
        by[off : off + n] = (i // g) * st
        off += n
    pl = np.stack([s, s, bx, by], 0)                     # [4, N_PAD]
    pl = np.concatenate([pl] * B_PER_CORE, 1)            # [4, 2*N_PAD]
    aux = pl.reshape(4, P, KPP).transpose(1, 0, 2)       # [P, 4, KPP]
    return np.ascontiguousarray(aux.astype(ml_dtypes.bfloat16))


def _host_in_maps(pred0, pred1, pred2):
    aux = _CACHE["consts"]
    pred0 = np.asarray(pred0, np.float32).reshape(B, -1, F)
    pred1 = np.asarray(pred1, np.float32).reshape(B, -1, F)
    pred2 = np.asarray(pred2, np.float32).reshape(B, -1, F)
    in_maps = []
    for core in range(N_CORES):
        flat = np.zeros((B_PER_CORE * N_PAD, F), np.float32)
        for j in range(B_PER_CORE):
            b = core * B_PER_CORE + j
            flat[j * N_PAD : j * N_PAD + N_REAL] = np.concatenate(
                [pred0[b], pred1[b], pred2[b]], axis=0
            )
        a = flat.reshape(P, KPP, F)                      # [p, k, field]
        predsT = np.empty((P, R, KPP), np.float32)
        predsT[:, 0, :] = a[:, :, 4]
        predsT[:, 1:, :] = a[:, :, 5:].transpose(0, 2, 1)
        in_maps.append(
            {
                "predsT": predsT.astype(ml_dtypes.float8_e4m3fn),
                "pa": np.ascontiguousarray(a[:, :, 0:4].transpose(0, 2, 1)),
                "aux": aux,
            }
        )
    return in_maps


def kernel(images, pred0, pred1, pred2):
    from concourse.bass_utils import run_bass_kernel_spmd

    if "nc" not in _CACHE:
        _CACHE["consts"] = _host_consts()
        _CACHE["nc"] = _build_nc()
    nc = _CACHE["nc"]

    in_maps = _host_in_maps(pred0, pred1, pred2)
    res = run_bass_kernel_spmd(nc, in_maps, list(range(N_CORES)))

    final = np.empty((B, N_REAL * C, 6), np.float32)
    v = final.reshape(B, N_REAL, C, 6)
    v[..., 4] = np.arange(C, dtype=np.float32)[None, None, :]
    for core, r in enumerate(res.results):
        # [P, C, KPP] -> per-image [N_REAL, C]; [P, 4, KPP] -> [N_REAL, 4]
        sc = (
            r["scores"].astype(np.float32)
            .reshape(B_PER_CORE, P // B_PER_CORE, C, KPP)
            .transpose(0, 1, 3, 2)
            .reshape(B_PER_CORE, N_PAD, C)
        )
        bx = (
            r["boxes"].astype(np.float32)
            .reshape(B_PER_CORE, P // B_PER_CORE, 4, KPP)
            .transpose(0, 1, 3, 2)
            .reshape(B_PER_CORE, N_PAD, 4)
        )
        for j in range(B_PER_CORE):
            b = core * B_PER_CORE + j
            v[b, :, :, 0:4] = bx[j, :N_REAL, None, :]
            v[b, :, :, 5] = sc[j, :N_REAL, :]
    return final
